# revision 51
# baseline (speedup 1.0000x reference)
"""AttentionBlock (GroupNorm + single-head self-attention + residual) on 8 TRN2
NeuronCores, data-parallel over batch; fp8(e4m3) DoubleRow matmuls (K=256 per
instruction) for the whole attention pipeline.

Shapes (hardcoded): x [32, 256, 32, 32], weights [256, 256], biases zero.
Each core processes 4 batch elements; no collectives.

Host-side marshalling: weights are pre-transposed/pre-scaled and cast to fp8 on
the host (pure layout/dtype prep); the weight folding matmuls, groupnorm,
projections, attention and normalization all run on-device.

Math folding with exact scale cancellation:
    wqT = 4 WQ^T, wkT = 4 WK^T, wvT = 4 WV^T, woQ = 4 Wo      (host, fp8)
    wqk = wqT^T wkT = 16 WQ WK^T = 256*scale*(WQ WK^T)        (device fold)
    wvo = wvT^T woQ = 16 WV Wo                                (device fold)
    g   = wqk^T h           = 256 * (scale WK WQ^T h)         [c', s]
    A^T = h^T g             = 256 * logits^T                  [t, s]
    E   = exp(A^T/256 - ln16) = exp(logits^T)/16              (ACT scale+bias)
    vw  = h^T wvo           = 16 * (h^T WV Wo)                [t, c_out]
    U'  = vw^T E            = unnormalized attn out (16/16 cancels)
    den = ones16^T E        = true softmax denominator (16 * E/16)
    y   = U' * (1/den) + x
The t-loop is split into two s-half phases so the PSUM accumulators
{U'_co0, U'_co1, den} fit 3 banks, double-buffered = 6 banks + 2 rotating.

Engine split: PE all matmuls; ACT the 64 Exp ops plus small psum
evacuations (gn group stats, half the vw evacs); DVE groupnorm stats +
rstd chain, h8 apply, the large psum evacuations, reciprocal and the
U'*(1/den) muls; GpSimd the +x residual adds and output DMA dispatch.

Scheduling notes (hard-won on traces):
 - pmm bufs=3 is mandatory: with 2, any evac copy in the rotation displaces
   the at->exp software pipeline by a full exp period.
 - den is single-buffered (pden): its only reader (reciprocal) runs at the
   start of each tail, freeing the bank before the next phase's den group.
 - groupnorm is split stats/gs/ch+h8 and prefetched 2-3 phases early so its
   tiny PE matmuls never couple the pmm rotation to a late DVE chain.
 - DMA descriptor-ring pressure is real: weights ship as ONE packed dma
   (2KB/partition lines); x batches stay on one queue so per-engine FIFO
   keeps x0 first.
 - Exec time is run-to-run noisy (HAM clock-gate phase, device thermal
   state): ~100us steady, up to ~119us on cold/slow runs.
"""

from contextlib import ExitStack

import numpy as np

B, C, HH, WW = 32, 256, 32, 32
S = HH * WW          # 1024 tokens
NCORES = 8
BLOC = B // NCORES   # 4 batch elements per core
P = 128
CT = C // P          # 2 channel tiles
TCH = S // P         # 8 t-chunks
NH = S // 512        # 2 s-halves of 512
UQ = TCH // 2        # 4 t-pair groups per phase (DoubleRow K=256)
GPT = P // 8         # 16 groups per channel tile (8 channels per group)
EPS = 1e-5
LN16 = 2.772588722239781
RSQRT_MAGIC_P1 = 0x5F3759DF + 1  # NOT(i>>1) + (K+1) == K - (i>>1)


def build_nc():
    import concourse.bass as bass  # noqa: F401
    import concourse.mybir as mybir
    import concourse.tile as tile
    from concourse import bacc

    f32 = mybir.dt.float32
    bf16 = mybir.dt.bfloat16
    fp8 = mybir.dt.float8e4
    i32 = mybir.dt.int32
    Alu = mybir.AluOpType
    Act = mybir.ActivationFunctionType
    DR = mybir.MatmulPerfMode.DoubleRow

    nc = bacc.Bacc("TRN2", target_bir_lowering=False, debug=False, num_devices=NCORES)

    x_ext = nc.dram_tensor("x", [BLOC, C, S], f32, kind="ExternalInput").ap()
    # all four (pre-transposed, pre-scaled fp8) weights packed host-side into
    # one [partition, name, ki, c] array -> ONE dma with 2KB/partition lines
    wall_ext = nc.dram_tensor("wall", [P, 4, CT, C], fp8,
                              kind="ExternalInput").ap()
    out_ext = nc.dram_tensor("out", [BLOC, C, S], f32, kind="ExternalOutput").ap()

    with tile.TileContext(nc) as tc, ExitStack() as ctx:
        consts = ctx.enter_context(tc.tile_pool(name="consts", bufs=1))
        sb = ctx.enter_context(tc.tile_pool(name="sb", bufs=2))
        small = ctx.enter_context(tc.tile_pool(name="small", bufs=4))
        pmm = ctx.enter_context(tc.tile_pool(name="pmm", bufs=3, space="PSUM"))
        pacc = ctx.enter_context(tc.tile_pool(name="pacc", bufs=2, space="PSUM"))
        # den is single-buffered: the reciprocal (first op of each tail) reads
        # it right after the last den matmul, so the next phase's den
        # accumulation can reuse the bank after a short WAR wait.
        pden = ctx.enter_context(tc.tile_pool(name="pden", bufs=1, space="PSUM"))

        # ---- PE warm-up junk matmuls: open the HAM clock gate before the
        # real stream arrives (~3.4us of PE activity needed).
        warm_sink = nc.dram_tensor("warm_sink", [P, 1], f32).ap()  # noqa: F841
        junk = consts.tile([P, 256], bf16, tag="junk", name="junk")
        nc.gpsimd.memset(junk[:, :], 0.001)
        warm_ps = pmm.tile([P, C], f32, tag="mm", name="warm_ps")
        for i in range(14):
            nc.tensor.matmul(warm_ps[:, :], junk[:, 0:P], junk[:, 0:C],
                             start=(i == 0), stop=(i == 13))

        # ---- input DMAs: x0 on sync, the packed weight block on scalar
        # (parallel descriptor generation, few large descriptors each).
        wall = consts.tile([P, 4, CT, C], fp8, tag="wall", name="wall")
        x_sb = []
        h8 = []
        for b in range(BLOC):
            x_sb.append(sb.tile([P, CT, S], f32, tag="x", bufs=BLOC, name=f"x{b}"))
            h8.append(sb.tile([P, CT, S], fp8, tag="h", bufs=BLOC, name=f"h{b}"))
        for ci in range(CT):
            for j in range(NH):
                nc.sync.dma_start(
                    out=x_sb[0][:, ci, j * 512:(j + 1) * 512],
                    in_=x_ext[0, ci * P:(ci + 1) * P, j * 512:(j + 1) * 512])
        nc.scalar.dma_start(out=wall[:, :, :, :], in_=wall_ext[:, :, :, :])

        # ---- weight folds: wqk = wqT^T wkT, wvo = wvT^T woQ (fp8 DoubleRow,
        # K=256 in one matmul per 128-wide output tile). Evacuations on ACT
        # so the DVE queue head stays free for groupnorm(0).
        wqk = consts.tile([P, CT, C], fp8, tag="wqk", name="wqk")
        wvo = consts.tile([P, CT, C], fp8, tag="wvo", name="wvo")
        for dst, li, ri in ((wqk, 0, 1), (wvo, 2, 3)):
            for j in range(CT):
                ps = pmm.tile([P, C], f32, tag="mm", name=f"fold{li}{j}")
                nc.tensor.matmul(ps[:, :], wall[:, li, :, j * P:(j + 1) * P],
                                 wall[:, ri, :, :], start=True, stop=True,
                                 perf_mode=DR)
                nc.scalar.copy(out=dst[:, j, :], in_=ps[:, :])

        # ---- group-average selector [128, 16]: sel[c, g] = (c//8 == g) / 8
        sel = consts.tile([P, GPT], bf16, tag="sel", name="sel")
        nc.gpsimd.memset(sel[:, :], 0.125)
        nc.gpsimd.affine_select(
            out=sel[:, :], in_=sel[:, :], compare_op=Alu.is_ge, fill=0.0,
            base=0, pattern=[[-8, GPT]], channel_multiplier=1,
        )
        nc.gpsimd.affine_select(
            out=sel[:, :], in_=sel[:, :], compare_op=Alu.is_ge, fill=0.0,
            base=7, pattern=[[8, GPT]], channel_multiplier=-1,
        )
        # broadcast-back selector [16, 128]: selT[g, c] = (c//8 == g)
        selT = consts.tile([GPT, P], bf16, tag="selT", name="selT")
        nc.gpsimd.memset(selT[:, :], 1.0)
        nc.gpsimd.affine_select(
            out=selT[:, :], in_=selT[:, :], compare_op=Alu.is_ge, fill=0.0,
            base=0, pattern=[[1, P]], channel_multiplier=-8,
        )
        nc.gpsimd.affine_select(
            out=selT[:, :], in_=selT[:, :], compare_op=Alu.is_ge, fill=0.0,
            base=7, pattern=[[-1, P]], channel_multiplier=8,
        )
        # all-16 fp8 [128, 2, 128] stationary operand for the den matmuls
        ones16 = consts.tile([P, CT, P], fp8, tag="ones16", name="ones16")
        nc.gpsimd.memset(ones16[:, :, :], 16.0)
        # exp bias vector: -ln(16) per partition
        nln16 = consts.tile([P, 1], f32, tag="nln16", name="nln16")
        nc.gpsimd.memset(nln16[:, :], -LN16)

        # x1-3 on the SAME sync queue as x0: per-DMA-engine FIFO order then
        # guarantees x0's chunks land first (a separate queue's descriptors
        # interleave and delay x0 by several us).
        for b in range(1, BLOC):
            for ci in range(CT):
                nc.sync.dma_start(out=x_sb[b][:, ci, :],
                                  in_=x_ext[b, ci * P:(ci + 1) * P, :])

        # second junk block: keep the PE (and its HAM clock gate) busy while
        # groupnorm(0) runs on DVE; real at-matmuls start right after.
        junk2 = pmm.tile([P, C], f32, tag="mm", name="junk2")
        for i in range(12):
            nc.tensor.matmul(junk2[:, :], junk[:, 0:P], junk[:, 0:C],
                             start=(i == 0), stop=(i == 11))



        # =============== per-batch emission helpers ===============
        # GroupNorm is split into three pieces (stats / group-reduce+rstd /
        # broadcast+apply) emitted in different phases so no piece's PE
        # matmul couples the at-pipeline's psum rotation to a late DVE
        # chain. Both channel tiles are packed in single [.,4] matmuls and
        # their psum is evacuated via ACT (fast, fixed-cadence queue).
        gn_state = {}

        def emit_stats(b):
            mvb4 = small.tile([P, 4], bf16, tag="mvb4", bufs=2, name=f"mvb{b}")
            for ci in range(CT):
                stats = small.tile([P, 2, 6], f32, tag="stats", name=f"st{b}{ci}")
                for j in range(2):
                    nc.vector.bn_stats(out=stats[:, j, :],
                                       in_=x_sb[b][:, ci, j * 512:(j + 1) * 512])
                mv = small.tile([P, 2], f32, tag="mv", name=f"mv{b}{ci}")
                nc.vector.bn_aggr(out=mv[:, :], in_=stats[:, :, :])
                # columns (2ci, 2ci+1) = (mean, E[x^2]) per channel
                nc.vector.tensor_copy(out=mvb4[:, 2 * ci:2 * ci + 1],
                                      in_=mv[:, 0:1])
                nc.vector.scalar_tensor_tensor(
                    out=mvb4[:, 2 * ci + 1:2 * ci + 2], in0=mv[:, 0:1],
                    scalar=mv[:, 0:1], in1=mv[:, 1:2],
                    op0=Alu.mult, op1=Alu.add)
            gn_state[b] = [mvb4]

        def emit_gs(b):
            (mvb4,) = gn_state[b]
            gs_ps = pmm.tile([GPT, 4], f32, tag="mm", name=f"gsp{b}")
            nc.tensor.matmul(gs_ps[:, :], sel[:, :], mvb4[:, :],
                             start=True, stop=True)
            gs4 = small.tile([GPT, 4], f32, tag="gs4", bufs=2, name=f"gs{b}")
            nc.scalar.copy(out=gs4[:, :], in_=gs_ps[:, :])
            # rstd = 1/sqrt(var+eps): bit-trick seed + Newton step on DVE
            nv = small.tile([GPT, CT], f32, tag="nv", name=f"nv{b}")
            for ci in range(CT):
                # nv = mean_g^2 - E[x^2]_g = -var_g  (fused in one op)
                nc.vector.scalar_tensor_tensor(
                    out=nv[:, ci:ci + 1], in0=gs4[:, 2 * ci:2 * ci + 1],
                    scalar=gs4[:, 2 * ci:2 * ci + 1],
                    in1=gs4[:, 2 * ci + 1:2 * ci + 2],
                    op0=Alu.mult, op1=Alu.subtract)
            vpack = small.tile([GPT, CT], f32, tag="vpack", name=f"vp{b}")
            nc.vector.tensor_scalar(out=vpack[:, :], in0=nv[:, :],
                                    scalar1=-1.0, scalar2=EPS,
                                    op0=Alu.mult, op1=Alu.add)
            x2 = small.tile([GPT, CT], f32, tag="x2", name=f"x2{b}")
            nc.vector.tensor_scalar_mul(out=x2[:, :], in0=vpack[:, :], scalar1=0.5)
            yr = small.tile([GPT, CT], f32, tag="yr", name=f"yr{b}")
            yri = yr[:, :].bitcast(i32)
            nc.vector.tensor_scalar(
                out=yri, in0=vpack[:, :].bitcast(i32), scalar1=1,
                scalar2=None, op0=Alu.arith_shift_right,
            )
            nc.vector.tensor_scalar(
                out=yri, in0=yri, scalar1=-1, scalar2=None, op0=Alu.bitwise_xor,
            )
            nc.vector.tensor_scalar(
                out=yri, in0=yri, scalar1=RSQRT_MAGIC_P1, scalar2=None, op0=Alu.add,
            )
            tmp = small.tile([GPT, CT], f32, tag="tmp", name=f"nr{b}")
            nc.vector.tensor_mul(out=tmp[:, :], in0=yr[:, :], in1=yr[:, :])
            nc.vector.tensor_mul(out=tmp[:, :], in0=tmp[:, :], in1=x2[:, :])
            nc.vector.tensor_scalar(
                out=tmp[:, :], in0=tmp[:, :], scalar1=-1.0, scalar2=1.5,
                op0=Alu.mult, op1=Alu.add,
            )
            nc.vector.tensor_mul(out=yr[:, :], in0=yr[:, :], in1=tmp[:, :])
            gsb4 = small.tile([GPT, 4], bf16, tag="gsb4", bufs=2, name=f"gsb{b}")
            for ci in range(CT):
                nc.vector.tensor_copy(out=gsb4[:, 2 * ci:2 * ci + 1],
                                      in_=gs4[:, 2 * ci:2 * ci + 1])
                nc.vector.tensor_copy(out=gsb4[:, 2 * ci + 1:2 * ci + 2],
                                      in_=yr[:, ci:ci + 1])
            gn_state[b] = [gsb4]

        def emit_ch_h8(b):
            (gsb4,) = gn_state.pop(b)
            ch_ps = pmm.tile([P, 4], f32, tag="mm", name=f"chp{b}")
            nc.tensor.matmul(ch_ps[:, :], selT[:, :], gsb4[:, :],
                             start=True, stop=True)
            ch4 = small.tile([P, 4], f32, tag="ch4", bufs=2, name=f"ch{b}")
            nc.scalar.copy(out=ch4[:, :], in_=ch_ps[:, :])
            # h = (x - mean) * rstd, cast straight to fp8 (DVE); j-major so
            # the first s-half (all channels) completes first
            for j in range(NH):
                for ci in range(CT):
                    sl = slice(j * 512, (j + 1) * 512)
                    nc.vector.tensor_scalar(
                        out=h8[b][:, ci, sl], in0=x_sb[b][:, ci, sl],
                        scalar1=ch4[:, 2 * ci:2 * ci + 1],
                        scalar2=ch4[:, 2 * ci + 1:2 * ci + 2],
                        op0=Alu.subtract, op1=Alu.mult,
                    )

        def emit_gn(b):
            emit_stats(b)
            emit_gs(b)
            emit_ch_h8(b)

        gv_state = {}

        def gv_chunks(b):
            """Projection matmuls for batch b as 8 lazily-emitted chunks
            (PE fillers inside the previous batch's second phase)."""
            gT = sb.tile([P, CT, S], fp8, tag="gT", name=f"gT{b}")
            v8 = sb.tile([P, TCH, C], fp8, tag="v8", name=f"v8{b}")
            gv_state[b] = (gT, v8)
            chunks = []
            for sh in range(NH):
                for co in range(CT):
                    def g_mm(co=co, sh=sh):
                        ps = pmm.tile([P, 512], f32, tag="mm", name=f"g{b}{co}{sh}")
                        nc.tensor.matmul(
                            ps[:, :], wqk[:, :, co * P:(co + 1) * P],
                            h8[b][:, :, sh * 512:(sh + 1) * 512],
                            start=True, stop=True, perf_mode=DR)
                        # sh=1 is consumed a full phase later: its evac can
                        # ride the ACT queue without head-blocking the
                        # next phase's at-matmuls
                        if sh == 1 and b > 0:
                            nc.scalar.copy(
                                out=gT[:, co, sh * 512:(sh + 1) * 512], in_=ps[:, :])
                        else:
                            nc.vector.tensor_copy(
                                out=gT[:, co, sh * 512:(sh + 1) * 512], in_=ps[:, :])
                    chunks.append(g_mm)
            # vw order (1,3,0,2): the ACT-evacuated pairs (u even) pop last,
            # landing in the ACT queue at the phase-boundary bubble instead
            # of stretching the mid-phase exp stream
            for u in (1, 3, 0, 2):
                def vw_mm(u=u):
                    ps = pmm.tile([P, 512], f32, tag="mm", name=f"vw{b}{u}")
                    nc.tensor.matmul(ps[:, 0:256],
                                     h8[b][:, :, (2 * u) * P:(2 * u + 1) * P],
                                     wvo[:, :, :], start=True, stop=False,
                                     perf_mode=DR)
                    nc.tensor.matmul(ps[:, 256:512],
                                     h8[b][:, :, (2 * u + 1) * P:(2 * u + 2) * P],
                                     wvo[:, :, :], start=False, stop=True,
                                     perf_mode=DR)
                    # alternate the evacuation between ACT and DVE to balance
                    if u % 2 == 0:
                        nc.scalar.copy(out=v8[:, 2 * u:2 * u + 2, :], in_=ps[:, :])
                    else:
                        nc.vector.tensor_copy(out=v8[:, 2 * u:2 * u + 2, :],
                                              in_=ps[:, :])
                chunks.append(vw_mm)
            return chunks

        # =============== phase machinery ===============
        # A phase is (b, sh): the full at->exp->ut/den pipeline for one
        # s-half of one batch. Accumulators: acc[:, 0/1, :] = U' co tiles,
        # acc[:, 2, :] = den; each is exactly one PSUM bank.

        phase_state = {}
        phase_at = {}

        def open_phase(p):
            b, sh = divmod(p, NH)
            e = sb.tile([P, TCH, 512], fp8, tag="expE", name=f"e{b}{sh}")
            acc = pacc.tile([P, CT, 512], f32, tag="acc", name=f"acc{b}{sh}")
            den = pden.tile([P, 512], f32, tag="den", name=f"den{b}{sh}")
            phase_state[p] = (e, acc, den)
            phase_at[p] = 0

        def emit_at_n(p, n):
            for _ in range(n):
                t = phase_at[p]
                if t < TCH:
                    at_mm(p, t)
                    phase_at[p] = t + 1

        def at_mm(p, t):
            b, sh = divmod(p, NH)
            e = phase_state[p][0]
            ps = pmm.tile([P, 512], f32, tag="mm", name=f"at{b}{sh}{t}")
            nc.tensor.matmul(ps[:, :], h8[b][:, :, t * P:(t + 1) * P],
                             gv_state[b][0][:, :, sh * 512:(sh + 1) * 512],
                             start=True, stop=True, perf_mode=DR)
            nc.scalar.activation(out=e[:, t, :], in_=ps[:, :], func=Act.Exp,
                                 bias=nln16[:, :], scale=1.0 / 256.0)

        def ut_den(p, u):
            b, sh = divmod(p, NH)
            e, acc, den = phase_state[p]
            esl = e[:, 2 * u:2 * u + 2, :]
            for co in range(CT):
                nc.tensor.matmul(acc[:, co, :],
                                 gv_state[b][1][:, 2 * u:2 * u + 2,
                                                co * P:(co + 1) * P],
                                 esl, start=(u == 0), stop=(u == UQ - 1),
                                 perf_mode=DR)
            nc.tensor.matmul(den[:, :], ones16[:, :, :], esl,
                             start=(u == 0), stop=(u == UQ - 1), perf_mode=DR)

        def emit_tail(p):
            """den -> 1/den -> U'*(1/den) (DVE), +x (GpSimd), DMA out (GpSimd).
            The last phase does the adds on DVE and the DMA on sync: the
            gpsimd ADD (1.3us) + queue hop would sit on the final drain."""
            b, sh = divmod(p, NH)
            last = p == BLOC * NH - 1
            _, acc, den = phase_state.pop(p)
            sl = slice(sh * 512, (sh + 1) * 512)
            ib = sb.tile([P, 512], f32, tag="ib", name=f"ib{b}{sh}")
            y1 = sb.tile([P, CT, 512], f32, tag="y1", name=f"y1{b}{sh}")
            y2 = sb.tile([P, CT, 512], f32, tag="y2", name=f"y2{b}{sh}")
            nc.vector.reciprocal_approx_fast(out=ib[:, :], in_=den[:, :])
            if last:
                # final drain: co0's +x and DMA ride GpSimd in parallel with
                # co1's mul/+x on DVE; DMA per co as soon as its add lands
                for co in range(CT):
                    nc.vector.tensor_mul(out=y1[:, co, :], in0=acc[:, co, :],
                                         in1=ib[:, :])
                    if co == 0:
                        nc.gpsimd.tensor_add(out=y2[:, co, :], in0=y1[:, co, :],
                                             in1=x_sb[b][:, co, sl])
                        nc.gpsimd.dma_start(
                            out=out_ext[b, co * P:(co + 1) * P, sl],
                            in_=y2[:, co, :])
                    else:
                        nc.vector.tensor_add(out=y2[:, co, :], in0=y1[:, co, :],
                                             in1=x_sb[b][:, co, sl])
                        nc.sync.dma_start(
                            out=out_ext[b, co * P:(co + 1) * P, sl],
                            in_=y2[:, co, :])
            else:
                for co in range(CT):
                    nc.vector.tensor_mul(out=y1[:, co, :], in0=acc[:, co, :],
                                         in1=ib[:, :])
                    nc.gpsimd.tensor_add(out=y2[:, co, :], in0=y1[:, co, :],
                                         in1=x_sb[b][:, co, sl])
                    nc.gpsimd.dma_start(out=out_ext[b, co * P:(co + 1) * P, sl],
                                        in_=y2[:, co, :])

        # =============== global emission schedule ===============
        # Software-pipelined across phases: the first two at-matmuls of
        # phase p+1 are emitted before the last ut/den group of phase p so
        # the ACT queue (the per-phase pacer) never drains. gn(b)+h8(b) is
        # emitted two+ phases before first use so the groupnorm chain never
        # convoys the DVE queue at a batch boundary; the gv(b+1) projection
        # chunks are spread across both phases of batch b as PE fillers.
        emit_gn(0)
        for f in gv_chunks(0):
            f()
        emit_gn(1)
        emit_stats(2)
        open_phase(0)
        emit_at_n(0, 2)
        NPH = BLOC * NH
        fillers = []
        for p in range(NPH):
            b, sh = divmod(p, NH)
            if sh == 0:
                if 2 <= b + 1 < BLOC:
                    emit_ch_h8(b + 1)
            else:
                if b + 3 < BLOC:
                    emit_stats(b + 3)
                if b + 2 < BLOC:
                    emit_gs(b + 2)
                if b + 1 < BLOC:
                    fillers = gv_chunks(b + 1)
            for u in range(UQ):
                if u < UQ - 1:
                    emit_at_n(p, 2)
                elif p + 1 < NPH:
                    open_phase(p + 1)
                    emit_at_n(p + 1, 2)
                # pop fillers front-loaded (4+4 in the first two u-steps) so
                # their evacuations queue on DVE/ACT well before the next
                # phase's at-matmuls need the results
                if fillers:
                    for _ in range(4):
                        if fillers:
                            fillers.pop(0)()
                ut_den(p, u)
            while fillers:
                fillers.pop(0)()
            emit_tail(p)

    nc.compile()
    return nc


_NC = None


def _get_nc():
    global _NC
    if _NC is None:
        _NC = build_nc()
    return _NC


def make_in_maps(x, WQ, WK, WV, Wo):
    import ml_dtypes

    x = np.ascontiguousarray(np.asarray(x, dtype=np.float32)).reshape(B, C, S)
    WQ, WK, WV, Wo = (np.asarray(w, dtype=np.float32) for w in (WQ, WK, WV, Wo))
    fp8 = ml_dtypes.float8_e4m3
    # wall[p, i, j, c] = W_i[128*j + p, c], W_i in (4 WQ^T, 4 WK^T, 4 WV^T, 4 Wo)
    wall = np.stack(
        [(4.0 * W).astype(fp8).reshape(CT, P, C).transpose(1, 0, 2)
         for W in (WQ.T, WK.T, WV.T, Wo)], axis=1)
    wall = np.ascontiguousarray(wall)
    return [
        {"x": x[i * BLOC:(i + 1) * BLOC], "wall": wall}
        for i in range(NCORES)
    ]


def run(in_maps, trace=False, **kw):
    from concourse.bass_utils import run_bass_kernel_spmd
    nc = _get_nc()
    return run_bass_kernel_spmd(nc, in_maps, core_ids=list(range(NCORES)),
                                trace=trace, **kw)


def kernel(x, WQ, WK, WV, Wo, bQ=None, bK=None, bV=None, bo=None, **_ignored):
    in_maps = make_in_maps(x, WQ, WK, WV, Wo)
    res = run(in_maps, trace=False)
    out = np.concatenate([res.results[i]["out"] for i in range(NCORES)], axis=0)
    return out.reshape(B, C, HH, WW).astype(np.float32)


# revision 52
# speedup vs baseline: 1.0063x; 1.0063x over previous
"""AttentionBlock (GroupNorm + single-head self-attention + residual) on 8 TRN2
NeuronCores, data-parallel over batch; fp8(e4m3) DoubleRow matmuls (K=256 per
instruction) for the whole attention pipeline.

Shapes (hardcoded): x [32, 256, 32, 32], weights [256, 256], biases zero.
Each core processes 4 batch elements; no collectives.

Host-side marshalling: weights are pre-transposed/pre-scaled and cast to fp8 on
the host (pure layout/dtype prep); the weight folding matmuls, groupnorm,
projections, attention and normalization all run on-device.

Math folding with exact scale cancellation:
    wqT = 4 WQ^T, wkT = 4 WK^T, wvT = 4 WV^T, woQ = 4 Wo      (host, fp8)
    wqk = wqT^T wkT = 16 WQ WK^T = 256*scale*(WQ WK^T)        (device fold)
    wvo = wvT^T woQ = 16 WV Wo                                (device fold)
    g   = wqk^T h           = 256 * (scale WK WQ^T h)         [c', s]
    A^T = h^T g             = 256 * logits^T                  [t, s]
    E   = exp(A^T/256 - ln16) = exp(logits^T)/16              (ACT scale+bias)
    vw  = h^T wvo           = 16 * (h^T WV Wo)                [t, c_out]
    U'  = vw^T E            = unnormalized attn out (16/16 cancels)
    den = ones16^T E        = true softmax denominator (16 * E/16)
    y   = U' * (1/den) + x
The t-loop is split into two s-half phases so the PSUM accumulators
{U'_co0, U'_co1, den} fit 3 banks, double-buffered = 6 banks + 2 rotating.

Engine split: PE all matmuls; ACT the 64 Exp ops plus small psum
evacuations (gn group stats, half the vw evacs); DVE groupnorm stats +
rstd chain, h8 apply, the large psum evacuations, reciprocal and the
U'*(1/den) muls; GpSimd the +x residual adds and output DMA dispatch.

Scheduling notes (hard-won on traces):
 - pmm bufs=3 is mandatory: with 2, any evac copy in the rotation displaces
   the at->exp software pipeline by a full exp period.
 - den is single-buffered (pden): its only reader (reciprocal) runs at the
   start of each tail, freeing the bank before the next phase's den group.
 - groupnorm is split stats/gs/ch+h8 and prefetched 2-3 phases early so its
   tiny PE matmuls never couple the pmm rotation to a late DVE chain.
 - DMA descriptor-ring pressure is real: weights ship as ONE packed dma
   (2KB/partition lines); x batches stay on one queue so per-engine FIFO
   keeps x0 first.
 - Exec time is run-to-run noisy (HAM clock-gate phase, device thermal
   state): ~100us steady, up to ~119us on cold/slow runs.
"""

from contextlib import ExitStack

import numpy as np

B, C, HH, WW = 32, 256, 32, 32
S = HH * WW          # 1024 tokens
NCORES = 8
BLOC = B // NCORES   # 4 batch elements per core
P = 128
CT = C // P          # 2 channel tiles
TCH = S // P         # 8 t-chunks
NH = S // 512        # 2 s-halves of 512
UQ = TCH // 2        # 4 t-pair groups per phase (DoubleRow K=256)
GPT = P // 8         # 16 groups per channel tile (8 channels per group)
EPS = 1e-5
LN16 = 2.772588722239781
RSQRT_MAGIC_P1 = 0x5F3759DF + 1  # NOT(i>>1) + (K+1) == K - (i>>1)


def build_nc():
    import concourse.bass as bass  # noqa: F401
    import concourse.mybir as mybir
    import concourse.tile as tile
    from concourse import bacc

    f32 = mybir.dt.float32
    bf16 = mybir.dt.bfloat16
    fp8 = mybir.dt.float8e4
    i32 = mybir.dt.int32
    Alu = mybir.AluOpType
    Act = mybir.ActivationFunctionType
    DR = mybir.MatmulPerfMode.DoubleRow

    nc = bacc.Bacc("TRN2", target_bir_lowering=False, debug=False, num_devices=NCORES)

    x_ext = nc.dram_tensor("x", [BLOC, C, S], f32, kind="ExternalInput").ap()
    # all four (pre-transposed, pre-scaled fp8) weights packed host-side into
    # one [partition, name, ki, c] array -> ONE dma with 2KB/partition lines
    wall_ext = nc.dram_tensor("wall", [P, 4, CT, C], fp8,
                              kind="ExternalInput").ap()
    out_ext = nc.dram_tensor("out", [BLOC, C, S], f32, kind="ExternalOutput").ap()

    with tile.TileContext(nc) as tc, ExitStack() as ctx:
        consts = ctx.enter_context(tc.tile_pool(name="consts", bufs=1))
        sb = ctx.enter_context(tc.tile_pool(name="sb", bufs=2))
        small = ctx.enter_context(tc.tile_pool(name="small", bufs=4))
        pmm = ctx.enter_context(tc.tile_pool(name="pmm", bufs=3, space="PSUM"))
        pacc = ctx.enter_context(tc.tile_pool(name="pacc", bufs=2, space="PSUM"))
        # den is single-buffered: the reciprocal (first op of each tail) reads
        # it right after the last den matmul, so the next phase's den
        # accumulation can reuse the bank after a short WAR wait.
        pden = ctx.enter_context(tc.tile_pool(name="pden", bufs=1, space="PSUM"))

        # ---- PE warm-up junk matmuls: open the HAM clock gate before the
        # real stream arrives (~3.4us of PE activity needed).
        warm_sink = nc.dram_tensor("warm_sink", [P, 1], f32).ap()  # noqa: F841
        junk = consts.tile([P, 256], bf16, tag="junk", name="junk")
        nc.gpsimd.memset(junk[:, :], 0.001)
        warm_ps = pmm.tile([P, C], f32, tag="mm", name="warm_ps")
        for i in range(14):
            nc.tensor.matmul(warm_ps[:, :], junk[:, 0:P], junk[:, 0:C],
                             start=(i == 0), stop=(i == 13))

        # ---- input DMAs: x0 on sync, the packed weight block on scalar
        # (parallel descriptor generation, few large descriptors each).
        wall = consts.tile([P, 4, CT, C], fp8, tag="wall", name="wall")
        x_sb = []
        h8 = []
        for b in range(BLOC):
            x_sb.append(sb.tile([P, CT, S], f32, tag="x", bufs=BLOC, name=f"x{b}"))
            h8.append(sb.tile([P, CT, S], fp8, tag="h", bufs=BLOC, name=f"h{b}"))
        for ci in range(CT):
            for j in range(NH):
                nc.sync.dma_start(
                    out=x_sb[0][:, ci, j * 512:(j + 1) * 512],
                    in_=x_ext[0, ci * P:(ci + 1) * P, j * 512:(j + 1) * 512])
        nc.scalar.dma_start(out=wall[:, :, :, :], in_=wall_ext[:, :, :, :])

        # ---- weight folds: wqk = wqT^T wkT, wvo = wvT^T woQ (fp8 DoubleRow,
        # K=256 in one matmul per 128-wide output tile). Evacuations on ACT
        # so the DVE queue head stays free for groupnorm(0).
        wqk = consts.tile([P, CT, C], fp8, tag="wqk", name="wqk")
        wvo = consts.tile([P, CT, C], fp8, tag="wvo", name="wvo")
        for dst, li, ri in ((wqk, 0, 1), (wvo, 2, 3)):
            for j in range(CT):
                ps = pmm.tile([P, C], f32, tag="mm", name=f"fold{li}{j}")
                nc.tensor.matmul(ps[:, :], wall[:, li, :, j * P:(j + 1) * P],
                                 wall[:, ri, :, :], start=True, stop=True,
                                 perf_mode=DR)
                nc.scalar.copy(out=dst[:, j, :], in_=ps[:, :])

        # ---- group-average selector [128, 16]: sel[c, g] = (c//8 == g) / 8
        sel = consts.tile([P, GPT], bf16, tag="sel", name="sel")
        nc.gpsimd.memset(sel[:, :], 0.125)
        nc.gpsimd.affine_select(
            out=sel[:, :], in_=sel[:, :], compare_op=Alu.is_ge, fill=0.0,
            base=0, pattern=[[-8, GPT]], channel_multiplier=1,
        )
        nc.gpsimd.affine_select(
            out=sel[:, :], in_=sel[:, :], compare_op=Alu.is_ge, fill=0.0,
            base=7, pattern=[[8, GPT]], channel_multiplier=-1,
        )
        # broadcast-back selector [16, 128]: selT[g, c] = (c//8 == g)
        selT = consts.tile([GPT, P], bf16, tag="selT", name="selT")
        nc.gpsimd.memset(selT[:, :], 1.0)
        nc.gpsimd.affine_select(
            out=selT[:, :], in_=selT[:, :], compare_op=Alu.is_ge, fill=0.0,
            base=0, pattern=[[1, P]], channel_multiplier=-8,
        )
        nc.gpsimd.affine_select(
            out=selT[:, :], in_=selT[:, :], compare_op=Alu.is_ge, fill=0.0,
            base=7, pattern=[[-1, P]], channel_multiplier=8,
        )
        # all-16 fp8 [128, 2, 128] stationary operand for the den matmuls
        ones16 = consts.tile([P, CT, P], fp8, tag="ones16", name="ones16")
        nc.gpsimd.memset(ones16[:, :, :], 16.0)
        # exp bias vector: -ln(16) per partition
        nln16 = consts.tile([P, 1], f32, tag="nln16", name="nln16")
        nc.gpsimd.memset(nln16[:, :], -LN16)

        # x1-3 on the SAME sync queue as x0: per-DMA-engine FIFO order then
        # guarantees x0's chunks land first (a separate queue's descriptors
        # interleave and delay x0 by several us).
        for b in range(1, BLOC):
            for ci in range(CT):
                nc.sync.dma_start(out=x_sb[b][:, ci, :],
                                  in_=x_ext[b, ci * P:(ci + 1) * P, :])

        # second junk block: keep the PE (and its HAM clock gate) busy while
        # groupnorm(0) runs on DVE; real at-matmuls start right after.
        junk2 = pmm.tile([P, C], f32, tag="mm", name="junk2")
        for i in range(12):
            nc.tensor.matmul(junk2[:, :], junk[:, 0:P], junk[:, 0:C],
                             start=(i == 0), stop=(i == 11))



        # =============== per-batch emission helpers ===============
        # GroupNorm is split into three pieces (stats / group-reduce+rstd /
        # broadcast+apply) emitted in different phases so no piece's PE
        # matmul couples the at-pipeline's psum rotation to a late DVE
        # chain. Both channel tiles are packed in single [.,4] matmuls and
        # their psum is evacuated via ACT (fast, fixed-cadence queue).
        gn_state = {}

        def emit_stats(b):
            mvb4 = small.tile([P, 4], bf16, tag="mvb4", bufs=2, name=f"mvb{b}")
            for ci in range(CT):
                stats = small.tile([P, 2, 6], f32, tag="stats", name=f"st{b}{ci}")
                for j in range(2):
                    nc.vector.bn_stats(out=stats[:, j, :],
                                       in_=x_sb[b][:, ci, j * 512:(j + 1) * 512])
                mv = small.tile([P, 2], f32, tag="mv", name=f"mv{b}{ci}")
                nc.vector.bn_aggr(out=mv[:, :], in_=stats[:, :, :])
                # columns (2ci, 2ci+1) = (mean, E[x^2]) per channel
                nc.vector.tensor_copy(out=mvb4[:, 2 * ci:2 * ci + 1],
                                      in_=mv[:, 0:1])
                nc.vector.scalar_tensor_tensor(
                    out=mvb4[:, 2 * ci + 1:2 * ci + 2], in0=mv[:, 0:1],
                    scalar=mv[:, 0:1], in1=mv[:, 1:2],
                    op0=Alu.mult, op1=Alu.add)
            gn_state[b] = [mvb4]

        def emit_gs(b):
            (mvb4,) = gn_state[b]
            gs_ps = pmm.tile([GPT, 4], f32, tag="mm", name=f"gsp{b}")
            nc.tensor.matmul(gs_ps[:, :], sel[:, :], mvb4[:, :],
                             start=True, stop=True)
            gs4 = small.tile([GPT, 4], f32, tag="gs4", bufs=2, name=f"gs{b}")
            nc.scalar.copy(out=gs4[:, :], in_=gs_ps[:, :])
            # rstd = 1/sqrt(var+eps): bit-trick seed + Newton step on DVE
            nv = small.tile([GPT, CT], f32, tag="nv", name=f"nv{b}")
            for ci in range(CT):
                # nv = mean_g^2 - E[x^2]_g = -var_g  (fused in one op)
                nc.vector.scalar_tensor_tensor(
                    out=nv[:, ci:ci + 1], in0=gs4[:, 2 * ci:2 * ci + 1],
                    scalar=gs4[:, 2 * ci:2 * ci + 1],
                    in1=gs4[:, 2 * ci + 1:2 * ci + 2],
                    op0=Alu.mult, op1=Alu.subtract)
            vpack = small.tile([GPT, CT], f32, tag="vpack", name=f"vp{b}")
            nc.vector.tensor_scalar(out=vpack[:, :], in0=nv[:, :],
                                    scalar1=-1.0, scalar2=EPS,
                                    op0=Alu.mult, op1=Alu.add)
            x2 = small.tile([GPT, CT], f32, tag="x2", name=f"x2{b}")
            nc.vector.tensor_scalar_mul(out=x2[:, :], in0=vpack[:, :], scalar1=0.5)
            yr = small.tile([GPT, CT], f32, tag="yr", name=f"yr{b}")
            yri = yr[:, :].bitcast(i32)
            nc.vector.tensor_scalar(
                out=yri, in0=vpack[:, :].bitcast(i32), scalar1=1,
                scalar2=None, op0=Alu.arith_shift_right,
            )
            nc.vector.tensor_scalar(
                out=yri, in0=yri, scalar1=-1, scalar2=None, op0=Alu.bitwise_xor,
            )
            nc.vector.tensor_scalar(
                out=yri, in0=yri, scalar1=RSQRT_MAGIC_P1, scalar2=None, op0=Alu.add,
            )
            tmp = small.tile([GPT, CT], f32, tag="tmp", name=f"nr{b}")
            nc.vector.tensor_mul(out=tmp[:, :], in0=yr[:, :], in1=yr[:, :])
            nc.vector.tensor_mul(out=tmp[:, :], in0=tmp[:, :], in1=x2[:, :])
            nc.vector.tensor_scalar(
                out=tmp[:, :], in0=tmp[:, :], scalar1=-1.0, scalar2=1.5,
                op0=Alu.mult, op1=Alu.add,
            )
            nc.vector.tensor_mul(out=yr[:, :], in0=yr[:, :], in1=tmp[:, :])
            gsb4 = small.tile([GPT, 4], bf16, tag="gsb4", bufs=2, name=f"gsb{b}")
            for ci in range(CT):
                nc.vector.tensor_copy(out=gsb4[:, 2 * ci:2 * ci + 1],
                                      in_=gs4[:, 2 * ci:2 * ci + 1])
                nc.vector.tensor_copy(out=gsb4[:, 2 * ci + 1:2 * ci + 2],
                                      in_=yr[:, ci:ci + 1])
            gn_state[b] = [gsb4]

        def emit_ch_h8(b):
            (gsb4,) = gn_state.pop(b)
            ch_ps = pmm.tile([P, 4], f32, tag="mm", name=f"chp{b}")
            nc.tensor.matmul(ch_ps[:, :], selT[:, :], gsb4[:, :],
                             start=True, stop=True)
            ch4 = small.tile([P, 4], f32, tag="ch4", bufs=2, name=f"ch{b}")
            nc.scalar.copy(out=ch4[:, :], in_=ch_ps[:, :])
            # h = (x - mean) * rstd, cast straight to fp8 (DVE); j-major so
            # the first s-half (all channels) completes first
            for j in range(NH):
                for ci in range(CT):
                    sl = slice(j * 512, (j + 1) * 512)
                    nc.vector.tensor_scalar(
                        out=h8[b][:, ci, sl], in0=x_sb[b][:, ci, sl],
                        scalar1=ch4[:, 2 * ci:2 * ci + 1],
                        scalar2=ch4[:, 2 * ci + 1:2 * ci + 2],
                        op0=Alu.subtract, op1=Alu.mult,
                    )

        def emit_gn(b):
            emit_stats(b)
            emit_gs(b)
            emit_ch_h8(b)

        gv_state = {}

        def gv_chunks(b):
            """Projection matmuls for batch b as 8 lazily-emitted chunks
            (PE fillers inside the previous batch's second phase)."""
            gT = sb.tile([P, CT, S], fp8, tag="gT", name=f"gT{b}")
            v8 = sb.tile([P, TCH, C], fp8, tag="v8", name=f"v8{b}")
            gv_state[b] = (gT, v8)
            chunks = []
            for sh in range(NH):
                for co in range(CT):
                    def g_mm(co=co, sh=sh):
                        ps = pmm.tile([P, 512], f32, tag="mm", name=f"g{b}{co}{sh}")
                        nc.tensor.matmul(
                            ps[:, :], wqk[:, :, co * P:(co + 1) * P],
                            h8[b][:, :, sh * 512:(sh + 1) * 512],
                            start=True, stop=True, perf_mode=DR)
                        # sh=1 is consumed a full phase later: its evac can
                        # ride the ACT queue without head-blocking the
                        # next phase's at-matmuls
                        if sh == 1 and b > 0:
                            nc.scalar.copy(
                                out=gT[:, co, sh * 512:(sh + 1) * 512], in_=ps[:, :])
                        else:
                            nc.vector.tensor_copy(
                                out=gT[:, co, sh * 512:(sh + 1) * 512], in_=ps[:, :])
                    chunks.append(g_mm)
            # vw order (1,3,0,2): the ACT-evacuated pairs (u even) pop last,
            # landing in the ACT queue at the phase-boundary bubble instead
            # of stretching the mid-phase exp stream
            for u in (1, 3, 0, 2):
                def vw_mm(u=u):
                    ps = pmm.tile([P, 512], f32, tag="mm", name=f"vw{b}{u}")
                    nc.tensor.matmul(ps[:, 0:256],
                                     h8[b][:, :, (2 * u) * P:(2 * u + 1) * P],
                                     wvo[:, :, :], start=True, stop=False,
                                     perf_mode=DR)
                    nc.tensor.matmul(ps[:, 256:512],
                                     h8[b][:, :, (2 * u + 1) * P:(2 * u + 2) * P],
                                     wvo[:, :, :], start=False, stop=True,
                                     perf_mode=DR)
                    # alternate the evacuation between ACT and DVE to balance
                    if u % 2 == 0:
                        nc.scalar.copy(out=v8[:, 2 * u:2 * u + 2, :], in_=ps[:, :])
                    else:
                        nc.vector.tensor_copy(out=v8[:, 2 * u:2 * u + 2, :],
                                              in_=ps[:, :])
                chunks.append(vw_mm)
            return chunks

        # =============== phase machinery ===============
        # A phase is (b, sh): the full at->exp->ut/den pipeline for one
        # s-half of one batch. Accumulators: acc[:, 0/1, :] = U' co tiles,
        # acc[:, 2, :] = den; each is exactly one PSUM bank.

        phase_state = {}
        phase_at = {}

        def open_phase(p):
            b, sh = divmod(p, NH)
            e = sb.tile([P, TCH, 512], fp8, tag="expE", name=f"e{b}{sh}")
            acc = pacc.tile([P, CT, 512], f32, tag="acc", name=f"acc{b}{sh}")
            den = pden.tile([P, 512], f32, tag="den", name=f"den{b}{sh}")
            phase_state[p] = (e, acc, den)
            phase_at[p] = 0

        def emit_at_n(p, n):
            for _ in range(n):
                t = phase_at[p]
                if t < TCH:
                    at_mm(p, t)
                    phase_at[p] = t + 1

        def at_mm(p, t):
            b, sh = divmod(p, NH)
            e = phase_state[p][0]
            ps = pmm.tile([P, 512], f32, tag="mm", name=f"at{b}{sh}{t}")
            nc.tensor.matmul(ps[:, :], h8[b][:, :, t * P:(t + 1) * P],
                             gv_state[b][0][:, :, sh * 512:(sh + 1) * 512],
                             start=True, stop=True, perf_mode=DR)
            nc.scalar.activation(out=e[:, t, :], in_=ps[:, :], func=Act.Exp,
                                 bias=nln16[:, :], scale=1.0 / 256.0)

        def ut_den(p, u):
            b, sh = divmod(p, NH)
            e, acc, den = phase_state[p]
            esl = e[:, 2 * u:2 * u + 2, :]
            for co in range(CT):
                nc.tensor.matmul(acc[:, co, :],
                                 gv_state[b][1][:, 2 * u:2 * u + 2,
                                                co * P:(co + 1) * P],
                                 esl, start=(u == 0), stop=(u == UQ - 1),
                                 perf_mode=DR)
            nc.tensor.matmul(den[:, :], ones16[:, :, :], esl,
                             start=(u == 0), stop=(u == UQ - 1), perf_mode=DR)

        def emit_tail(p):
            """den -> 1/den -> U'*(1/den) (DVE), +x (GpSimd), DMA out (GpSimd).
            The last phase does the adds on DVE and the DMA on sync: the
            gpsimd ADD (1.3us) + queue hop would sit on the final drain."""
            b, sh = divmod(p, NH)
            last = p == BLOC * NH - 1
            _, acc, den = phase_state.pop(p)
            sl = slice(sh * 512, (sh + 1) * 512)
            ib = sb.tile([P, 512], f32, tag="ib", name=f"ib{b}{sh}")
            y1 = sb.tile([P, CT, 512], f32, tag="y1", name=f"y1{b}{sh}")
            y2 = sb.tile([P, CT, 512], f32, tag="y2", name=f"y2{b}{sh}")
            nc.vector.reciprocal_approx_fast(out=ib[:, :], in_=den[:, :])
            if last:
                # final drain: co0's +x and DMA ride GpSimd in parallel with
                # co1's mul/+x on DVE; DMA per co as soon as its add lands
                for co in range(CT):
                    nc.vector.tensor_mul(out=y1[:, co, :], in0=acc[:, co, :],
                                         in1=ib[:, :])
                    if co == 0:
                        nc.gpsimd.tensor_add(out=y2[:, co, :], in0=y1[:, co, :],
                                             in1=x_sb[b][:, co, sl])
                        nc.gpsimd.dma_start(
                            out=out_ext[b, co * P:(co + 1) * P, sl],
                            in_=y2[:, co, :])
                    else:
                        nc.vector.tensor_add(out=y2[:, co, :], in0=y1[:, co, :],
                                             in1=x_sb[b][:, co, sl])
                        nc.sync.dma_start(
                            out=out_ext[b, co * P:(co + 1) * P, sl],
                            in_=y2[:, co, :])
            else:
                for co in range(CT):
                    nc.vector.tensor_mul(out=y1[:, co, :], in0=acc[:, co, :],
                                         in1=ib[:, :])
                    nc.gpsimd.tensor_add(out=y2[:, co, :], in0=y1[:, co, :],
                                         in1=x_sb[b][:, co, sl])
                    nc.gpsimd.dma_start(out=out_ext[b, co * P:(co + 1) * P, sl],
                                        in_=y2[:, co, :])

        # =============== global emission schedule ===============
        # Software-pipelined across phases: the first two at-matmuls of
        # phase p+1 are emitted before the last ut/den group of phase p so
        # the ACT queue (the per-phase pacer) never drains. gn(b)+h8(b) is
        # emitted two+ phases before first use so the groupnorm chain never
        # convoys the DVE queue at a batch boundary; the gv(b+1) projection
        # chunks are spread across both phases of batch b as PE fillers.
        emit_gn(0)
        for f in gv_chunks(0):
            f()
        emit_gn(1)
        emit_stats(2)
        open_phase(0)
        emit_at_n(0, 2)
        NPH = BLOC * NH
        fillers = []
        for p in range(NPH):
            b, sh = divmod(p, NH)
            if sh == 0:
                if 2 <= b + 1 < BLOC:
                    emit_ch_h8(b + 1)
            else:
                if b + 3 < BLOC:
                    emit_stats(b + 3)
                if b + 2 < BLOC:
                    emit_gs(b + 2)
                if b + 1 < BLOC:
                    fillers = gv_chunks(b + 1)
            for u in range(UQ):
                if u < UQ - 1:
                    emit_at_n(p, 2)
                elif p + 1 < NPH:
                    open_phase(p + 1)
                    emit_at_n(p + 1, 2)
                if fillers:
                    fillers.pop(0)()
                    if fillers:
                        fillers.pop(0)()
                ut_den(p, u)
            while fillers:
                fillers.pop(0)()
            emit_tail(p)

    nc.compile()
    return nc


_NC = None


def _get_nc():
    global _NC
    if _NC is None:
        _NC = build_nc()
    return _NC


def make_in_maps(x, WQ, WK, WV, Wo):
    import ml_dtypes

    x = np.ascontiguousarray(np.asarray(x, dtype=np.float32)).reshape(B, C, S)
    WQ, WK, WV, Wo = (np.asarray(w, dtype=np.float32) for w in (WQ, WK, WV, Wo))
    fp8 = ml_dtypes.float8_e4m3
    # wall[p, i, j, c] = W_i[128*j + p, c], W_i in (4 WQ^T, 4 WK^T, 4 WV^T, 4 Wo)
    wall = np.stack(
        [(4.0 * W).astype(fp8).reshape(CT, P, C).transpose(1, 0, 2)
         for W in (WQ.T, WK.T, WV.T, Wo)], axis=1)
    wall = np.ascontiguousarray(wall)
    return [
        {"x": x[i * BLOC:(i + 1) * BLOC], "wall": wall}
        for i in range(NCORES)
    ]


def run(in_maps, trace=False, **kw):
    from concourse.bass_utils import run_bass_kernel_spmd
    nc = _get_nc()
    return run_bass_kernel_spmd(nc, in_maps, core_ids=list(range(NCORES)),
                                trace=trace, **kw)


def kernel(x, WQ, WK, WV, Wo, bQ=None, bK=None, bV=None, bo=None, **_ignored):
    in_maps = make_in_maps(x, WQ, WK, WV, Wo)
    res = run(in_maps, trace=False)
    out = np.concatenate([res.results[i]["out"] for i in range(NCORES)], axis=0)
    return out.reshape(B, C, HH, WW).astype(np.float32)
